# revision 1
# baseline (speedup 1.0000x reference)
"""Trainium2 Bass kernel for nn_DenseNetCmaxGatedB2 (gated pooling block).

Rewritten from the DVE/ACT-only baseline to use all engines:
  out = maxpool3x3s2(x) * (dwconv_s2(x, maxgate) + mb) + n1 + g*(n0 - n1)
  n0  = p1 + g0*d01;  n1 = p3 + g1*d23    (d01/d23 = difference-weight convs)
  g0/g1/g = sigmoid(conv + bias);  node conv g is stride-1 on n0.

Knobs:
  PE_CONVS : which stride-2 convs run on the TensorE (diagonal matmuls)
  TILE4    : use 4x (32x32) diagonal tile_position matmuls (hides LDWEIGHTS)
  cm + any conv not in PE_CONVS runs on DVE (tensor_scalar 4x products +
  tensor_tensor 2x adds, products alternating with ACT).
"""

import contextlib
import sys

sys.path.insert(0, "/opt/trn_rl_repo")

import numpy as np

import concourse.bass as bass  # noqa: E402,F401
import concourse.mybir as mybir  # noqa: E402
from concourse import bacc  # noqa: E402
from concourse.tile import TileContext  # noqa: E402
from concourse.bass_utils import run_bass_kernel_spmd  # noqa: E402

N_CORES = 8
B, C, H = 16, 256, 128
HO = H // 2
BS = B // N_CORES
F32 = mybir.dt.float32
BF16 = mybir.dt.bfloat16
AF = mybir.ActivationFunctionType
OP = mybir.AluOpType

ALL_CONVS = ["g0", "d01", "p1", "g1", "d23", "p3"]
PE_CONVS = ["g0", "d01", "p1", "g1", "d23"]  # p3 -> DVE
PROD_ACT = {0, 2, 6, 8}  # tap indices whose products go to ACT
TILE4 = True
BIAS_KEYS = ["mb", "g0", "d01", "p1", "g1", "d23", "p3", "gc"]
def _sc_convs():
    return ["mb"] + [k for k in ALL_CONVS if k not in PE_CONVS]
TAPS9 = [(di, dj) for di in range(3) for dj in range(3)]


def _build(reps=1):
    SC_CONVS = _sc_convs()
    npe = len(PE_CONVS)
    nsc = len(SC_CONVS)
    nc = bacc.Bacc("TRN2", target_bir_lowering=False, debug=False, num_devices=N_CORES)

    x_d = nc.dram_tensor("x", [BS, C, H * H], F32, kind="ExternalInput")
    wdiag_d = nc.dram_tensor(
        "wdiag", [128, 2 * npe * 9 * 128], F32, kind="ExternalInput"
    )
    wsc_d = nc.dram_tensor("wsc", [C, nsc * 9], F32, kind="ExternalInput")
    bias_d = nc.dram_tensor("biases", [C, len(BIAS_KEYS)], F32, kind="ExternalInput")
    out_d = nc.dram_tensor("out", [BS, C, HO * HO], F32, kind="ExternalOutput")

    V = nc.vector
    S = nc.scalar
    G = nc.gpsimd

    with TileContext(nc) as tc:
        with contextlib.ExitStack() as ctx:
            wp = ctx.enter_context(tc.tile_pool(name="w", bufs=1))
            xhp = ctx.enter_context(tc.tile_pool(name="xh", bufs=2))
            pp = ctx.enter_context(tc.tile_pool(name="pp", bufs=1))
            evp = ctx.enter_context(tc.tile_pool(name="ev", bufs=2))
            n0p = ctx.enter_context(tc.tile_pool(name="n0p", bufs=2))
            mid = ctx.enter_context(tc.tile_pool(name="mid", bufs=1))
            tmp = ctx.enter_context(tc.tile_pool(name="tmp", bufs=2))
            ps = ctx.enter_context(tc.tile_pool(name="ps", bufs=2, space="PSUM"))

            # ---- weights (resident)
            w9 = wp.tile([128, 2, npe, 9, 128], BF16, tag="w9")
            G.dma_start(w9[:].rearrange("p a b c d -> p (a b c d)"), wdiag_d[:, :])
            wsc = wp.tile([128, 2, nsc, 9], F32, tag="wsc")
            bb = wp.tile([128, 2, len(BIAS_KEYS)], F32, tag="bb")
            for cb in range(2):
                sl = slice(cb * 128, (cb + 1) * 128)
                nc.sync.dma_start(
                    wsc[:, cb, :, :].rearrange("p a b -> p (a b)"), wsc_d[sl, :]
                )
                nc.sync.dma_start(bb[:, cb, :], bias_d[sl, :])

            def bias_ap(cb, key):
                k = BIAS_KEYS.index(key)
                return bb[:, cb, k : k + 1]

            state = {}

            def stage_a(b, cb):
                sl = slice(cb * 128, (cb + 1) * 128)

                ee = pp.tile([128, 64, 64], BF16, tag="ee", name="ee")
                eo = pp.tile([128, 64, 64], BF16, tag="eo", name="eo")
                oe = pp.tile([128, 64, 64], BF16, tag="oe", name="oe")
                oo = pp.tile([128, 64, 64], BF16, tag="oo", name="oo")
                ez = pp.tile([128, 64, 66], BF16, tag="ez", name="ez")
                oz = pp.tile([128, 64, 66], BF16, tag="oz", name="oz")
                G.memset(ez[:, :, 0:1], 0)
                G.memset(oz[:, :, 0:1], 0)

                for h in range(4):
                    Xh = xhp.tile([128, 32, 128], BF16, tag="Xh", name="Xh")
                    G.dma_start(
                        Xh[:].rearrange("p a b -> p (a b)"),
                        x_d[b, sl, h * 4096 : (h + 1) * 4096],
                    )
                    hs = slice(16 * h, 16 * h + 16)
                    S.copy(ee[:, hs, :], Xh[:, 0:32:2, 0:128:2])
                    S.copy(eo[:, hs, :], Xh[:, 0:32:2, 1:128:2])
                    S.copy(oe[:, hs, :], Xh[:, 1:32:2, 0:128:2])
                    S.copy(oo[:, hs, :], Xh[:, 1:32:2, 1:128:2])
                    G.tensor_copy(ez[:, hs, 1:65], Xh[:, 0:32:2, 1:128:2])
                    G.tensor_copy(oz[:, hs, 1:65], Xh[:, 1:32:2, 1:128:2])

                def plane_view(di, dj):
                    if di == 1:
                        return {0: ez, 1: ee, 2: eo}[dj]
                    return {0: oz, 1: oe, 2: oo}[dj]

                def ev_tile(key):
                    if key in ("g0", "g1"):
                        return evp.tile([128, 64, 64], BF16, tag="g", name=key)
                    if key in ("d01", "d23"):
                        return evp.tile([128, 64, 64], BF16, tag="d", name=key)
                    if key == "p1":
                        return evp.tile(
                            [128, 64, 64], BF16, tag="p1", name=key, bufs=1
                        )
                    return evp.tile([128, 64, 64], BF16, tag="p3", name=key)

                ev_tiles = {}

                # ---- PE convs
                for key in PE_CONVS:
                    cvi = PE_CONVS.index(key)
                    dst = ev_tile(key)
                    ev_tiles[key] = dst
                    func = AF.Sigmoid if key in ("g0", "g1") else AF.Identity
                    for h in range(2):
                        acc = ps.tile([128, 2048], F32, tag="ph", name="ph")
                        for c in range(4):
                            r0c = 32 * h + 8 * c
                            for t, (di, dj) in enumerate(TAPS9):
                                pl = plane_view(di, dj)
                                ro = -1 if di == 0 else 0
                                r0, nr, o0 = r0c, 8, 0
                                if di == 0 and r0c == 0:
                                    r0, nr, o0 = 1, 7, 64
                                if TILE4:
                                    for g in range(4):
                                        gs = slice(32 * g, 32 * g + 32)
                                        nc.tensor.matmul(
                                            acc[gs, 512 * c + o0 : 512 * c + 512],
                                            w9[gs, cb, cvi, t, gs],
                                            pl[gs, r0 + ro : r0 + ro + nr, 0:64],
                                            start=(t == 0),
                                            stop=(t == 8),
                                            tile_position=(32 * g, 32 * g),
                                        )
                                else:
                                    nc.tensor.matmul(
                                        acc[:, 512 * c + o0 : 512 * c + 512],
                                        w9[:, cb, cvi, t, :],
                                        pl[:, r0 + ro : r0 + ro + nr, 0:64],
                                        start=(t == 0),
                                        stop=(t == 8),
                                    )
                        S.activation(
                            dst[:, 32 * h : 32 * h + 32, :],
                            acc[:].rearrange("p (r c) -> p r c", r=32),
                            func,
                            bias=bias_ap(cb, key),
                        )

                # ---- DVE convs (cm + any conv not on PE)
                def wsc_s(key, t):
                    k = _sc_convs().index(key)
                    return wsc[:, cb, k, t : t + 1]

                def dve_conv(key, dst, bias_key):
                    V.tensor_scalar(
                        dst[:], ee[:], wsc_s(key, 4), bias_ap(cb, bias_key),
                        OP.mult, OP.add,
                    )
                    for t, (di, dj) in enumerate(TAPS9):
                        if di == 1 and dj == 1:
                            continue
                        pl = plane_view(di, dj)
                        i0 = 1 if di == 0 else 0
                        pin = pl[:, 0 : 64 - i0, 0:64]
                        po = dst[:, i0:64, :]
                        t_ = tmp.tile([128, 64, 64], BF16, tag="t", bufs=2, name="t")
                        tv = t_[:, 0 : 64 - i0, :]
                        if t in PROD_ACT:
                            S.mul(tv, pin, wsc_s(key, t))
                        else:
                            V.tensor_scalar(tv, pin, wsc_s(key, t), None, OP.mult)
                        V.tensor_tensor(po, po, tv, OP.add)

                cm = mid.tile([128, 64, 64], BF16, tag="cm", name="cm", bufs=2)
                dve_conv("mb", cm, "mb")
                for key in ALL_CONVS:
                    if key in PE_CONVS:
                        continue
                    dst = ev_tile(key)
                    ev_tiles[key] = dst
                    dve_conv(key, dst, key)
                    if key in ("g0", "g1"):
                        S.activation(dst[:], dst[:], AF.Sigmoid)

                # ---- separable maxpool -> mp
                mp = mid.tile([128, 64, 64], BF16, tag="mp", name="mp")
                m1e = tmp.tile([128, 64, 64], BF16, tag="t", bufs=2, name="m1e")
                m1o = tmp.tile([128, 64, 64], BF16, tag="t", bufs=2, name="m1o")
                V.tensor_tensor(m1e[:], ee[:], eo[:], OP.max)
                V.tensor_tensor(m1o[:], oe[:], oo[:], OP.max)
                V.tensor_tensor(
                    m1e[:, :, 1:64], m1e[:, :, 1:64], ez[:, :, 1:64], OP.max
                )
                V.tensor_tensor(
                    m1o[:, :, 1:64], m1o[:, :, 1:64], oz[:, :, 1:64], OP.max
                )
                V.tensor_tensor(mp[:], m1e[:], m1o[:], OP.max)
                V.tensor_tensor(
                    mp[:, 1:64, :], mp[:, 1:64, :], m1o[:, 0:63, :], OP.max
                )

                V.tensor_tensor(cm[:], cm[:], mp[:], OP.mult)  # mpcm

                n0z = n0p.tile([128, 66, 68], BF16, tag="n0z", name="n0z")
                G.memset(n0z[:, 0:1, :], 0)
                G.memset(n0z[:, 65:66, :], 0)
                G.memset(n0z[:, 1:65, 1:2], 0)
                G.memset(n0z[:, 1:65, 66:67], 0)
                e01 = ev_tiles["d01"]
                V.tensor_tensor(e01[:], e01[:], ev_tiles["g0"][:], OP.mult)
                V.tensor_tensor(n0z[:, 1:65, 2:66], ev_tiles["p1"][:], e01[:], OP.add)
                e23 = ev_tiles["d23"]
                V.tensor_tensor(e23[:], e23[:], ev_tiles["g1"][:], OP.mult)
                n1 = ev_tiles["p3"]
                V.tensor_tensor(n1[:], n1[:], e23[:], OP.add)

                state[(b, cb)] = dict(n0z=n0z, n1=n1, mpcm=cm)

            def stage_b(b, cb):
                sl = slice(cb * 128, (cb + 1) * 128)
                st = state.pop((b, cb))
                n0z, n1, mpcm = st["n0z"], st["n1"], st["mpcm"]

                g1i = PE_CONVS.index("g1")
                gc = evp.tile([128, 64, 64], BF16, tag="g", name="gc")
                for h in range(2):
                    acc = ps.tile([128, 2048], F32, tag="ph", name="ph")
                    for c in range(4):
                        r0 = 32 * h + 8 * c
                        for t, (di, dj) in enumerate(TAPS9):
                            if TILE4:
                                for g in range(4):
                                    gs = slice(32 * g, 32 * g + 32)
                                    nc.tensor.matmul(
                                        acc[gs, 512 * c : 512 * c + 512],
                                        w9[gs, cb, g1i, t, gs],
                                        n0z[gs, r0 + di : r0 + di + 8, dj + 1 : dj + 65],
                                        start=(t == 0),
                                        stop=(t == 8),
                                        tile_position=(32 * g, 32 * g),
                                    )
                            else:
                                nc.tensor.matmul(
                                    acc[:, 512 * c : 512 * c + 512],
                                    w9[:, cb, g1i, t, :],
                                    n0z[:, r0 + di : r0 + di + 8, dj + 1 : dj + 65],
                                    start=(t == 0),
                                    stop=(t == 8),
                                )
                    S.activation(
                        gc[:, 32 * h : 32 * h + 32, :],
                        acc[:].rearrange("p (r c) -> p r c", r=32),
                        AF.Sigmoid,
                        bias=bias_ap(cb, "gc"),
                    )

                n0 = n0z[:, 1:65, 2:66]
                d = tmp.tile([128, 64, 64], BF16, tag="t", bufs=2, name="d")
                V.tensor_tensor(d[:], n0, n1[:], OP.subtract)
                V.tensor_tensor(d[:], d[:], gc[:], OP.mult)
                V.tensor_tensor(d[:], d[:], n1[:], OP.add)
                V.tensor_tensor(d[:], d[:], mpcm[:], OP.add)
                G.dma_start(out_d[b, sl, :], d[:].rearrange("p a b -> p (a b)"))

            planes = [(b, cb) for b in range(BS) for cb in range(2)]
            rep_ctx = tc.For_i(0, reps, 1) if reps > 1 else contextlib.nullcontext()
            with rep_ctx:
                stage_a(*planes[0])
                for i in range(1, len(planes)):
                    stage_a(*planes[i])
                    stage_b(*planes[i - 1])
                stage_b(*planes[-1])

    nc.compile()
    return nc


_NC_CACHE = {}


def _get_nc(reps=1):
    key = (tuple(PE_CONVS), TILE4, reps)
    if key not in _NC_CACHE:
        _NC_CACHE[key] = _build(reps)
    return _NC_CACHE[key]


def _prep_weights(maxgate, mb, pconvs, pbs, pgates, gbs):
    SC_CONVS = _sc_convs()
    npe = len(PE_CONVS)
    mg = np.asarray(maxgate, np.float32).reshape(C, 9)
    pc = np.asarray(pconvs, np.float32).reshape(C, 9, 4)
    pg = np.asarray(pgates, np.float32).reshape(C, 9, 3)
    pbs = np.asarray(pbs, np.float32)
    gbs = np.asarray(gbs, np.float32)
    mb = np.asarray(mb, np.float32).reshape(C)

    wconvs = {
        "mb": mg,
        "g0": pg[:, :, 0],
        "d01": pc[:, :, 0] - pc[:, :, 1],
        "p1": pc[:, :, 1],
        "g1": pg[:, :, 2],
        "d23": pc[:, :, 2] - pc[:, :, 3],
        "p3": pc[:, :, 3],
    }
    wd = np.zeros((128, 2, npe, 9, 128), np.float32)
    idx = np.arange(128)
    for cb in range(2):
        for cvi, key in enumerate(PE_CONVS):
            wd[idx, cb, cvi, :, idx] = wconvs[key][cb * 128 : (cb + 1) * 128, :]
    wsc = np.stack([wconvs[k] for k in SC_CONVS], axis=1)  # [C, nsc, 9]
    biases = np.stack(
        [
            mb,
            gbs[:, 0],
            pbs[:, 0] - pbs[:, 1],
            pbs[:, 1],
            gbs[:, 1],
            pbs[:, 2] - pbs[:, 3],
            pbs[:, 3],
            gbs[:, 2],
        ],
        axis=1,
    ).astype(np.float32)
    return (
        wd.reshape(128, 2 * npe * 9 * 128),
        wsc.reshape(C, len(SC_CONVS) * 9).astype(np.float32),
        biases,
    )


def _in_maps(x, maxgate, mb, pconvs, pbs, pgates, gbs):
    x = np.ascontiguousarray(np.asarray(x, np.float32))
    wd, wsc, biases = _prep_weights(maxgate, mb, pconvs, pbs, pgates, gbs)
    maps = []
    for i in range(N_CORES):
        maps.append(
            dict(
                x=x[i * BS : (i + 1) * BS].reshape(BS, C, H * H),
                wdiag=wd,
                wsc=wsc,
                biases=biases,
            )
        )
    return maps


def kernel(x, maxgate, mb, pconvs, pbs, pgates, gbs):
    nc = _get_nc(1)
    maps = _in_maps(x, maxgate, mb, pconvs, pbs, pgates, gbs)
    res = run_bass_kernel_spmd(nc, maps, list(range(N_CORES)))
    return np.concatenate(
        [r["out"].reshape(BS, C, HO, HO) for r in res.results], axis=0
    )



# revision 8
# speedup vs baseline: 1.0796x; 1.0796x over previous
"""Trainium2 Bass kernel for nn_DenseNetCmaxGatedB2 (gated pooling block).

v2: pipelined rewrite of the diagonal-matmul baseline.
  out = maxpool3x3s2(x) * (dwconv_s2(x, maxgate) + mb) + n1 + g*(n0 - n1)
  n0  = p1 + g0*d01;  n1 = p3 + g1*d23    (d01/d23 = difference-weight convs)
  g0/g1/g = sigmoid(conv + bias);  node conv g is stride-1 on n0 (weights = g1
  weights per the reference's pgates[...,2] reuse, bias gbs[:,2]).

Key changes vs v1:
  - phase-plane pool double-buffered (bufs=2): plane i+1's deinterleave
    overlaps plane i's compute (v1 serialized planes on this pool).
  - ez/oz shifted tiles dropped; column-shift taps handled by ACT products
    into a zero-padded tmp (ACT is stride/alignment agnostic), keeping all
    DVE adds in 2x bf16 mode.  Maxpool pays 2 unaligned (1x) ops instead.
  - every PE conv is evicted promptly by ACT (bias/sigmoid fused), so PSUM
    never backs up the PE.
  - cm/p3 convs on DVE as tensor_scalar(4x) products + tensor_tensor(2x)
    adds, with 4 of 9 products on ACT.
"""

import contextlib
import sys

sys.path.insert(0, "/opt/trn_rl_repo")

import numpy as np

import concourse.bass as bass  # noqa: E402,F401
import concourse.mybir as mybir  # noqa: E402
from concourse import bacc  # noqa: E402
from concourse.tile import TileContext  # noqa: E402
from concourse.bass_utils import run_bass_kernel_spmd  # noqa: E402

N_CORES = 8
B, C, H = 16, 256, 128
HO = H // 2
BS = B // N_CORES
F32 = mybir.dt.float32
BF16 = mybir.dt.bfloat16
AF = mybir.ActivationFunctionType
OP = mybir.AluOpType

PE_CONVS = ["g0", "d01", "p1", "g1", "d23"]
SC_CONVS = ["mb", "p3"]
BIAS_KEYS = ["mb", "g0", "d01", "p1", "g1", "d23", "p3", "gc"]
TAPS9 = [(di, dj) for di in range(3) for dj in range(3)]
NH = 8  # x row-chunks per plane (16 rows each)


def _build(reps=1):
    npe = len(PE_CONVS)
    nsc = len(SC_CONVS)
    nc = bacc.Bacc("TRN2", target_bir_lowering=False, debug=False, num_devices=N_CORES)

    x_d = nc.dram_tensor("x", [BS, C, H * H], F32, kind="ExternalInput")
    wdiag_d = nc.dram_tensor(
        "wdiag", [128, 2 * npe * 9 * 128], F32, kind="ExternalInput"
    )
    wsc_d = nc.dram_tensor("wsc", [C, nsc * 9], F32, kind="ExternalInput")
    bias_d = nc.dram_tensor("biases", [C, len(BIAS_KEYS)], F32, kind="ExternalInput")
    out_d = nc.dram_tensor("out", [BS, C, HO * HO], F32, kind="ExternalOutput")

    V = nc.vector
    S = nc.scalar
    G = nc.gpsimd

    with TileContext(nc) as tc:
        with contextlib.ExitStack() as ctx:
            wp = ctx.enter_context(tc.tile_pool(name="w", bufs=1))
            xhp = ctx.enter_context(tc.tile_pool(name="xh", bufs=2))
            pp = ctx.enter_context(tc.tile_pool(name="pp", bufs=2))
            gp = ctx.enter_context(tc.tile_pool(name="gp", bufs=1))
            stp = ctx.enter_context(tc.tile_pool(name="stp", bufs=2))
            dtp = ctx.enter_context(tc.tile_pool(name="dtp", bufs=2))
            tmp = ctx.enter_context(tc.tile_pool(name="tmp", bufs=2))
            ps = ctx.enter_context(tc.tile_pool(name="ps", bufs=2, space="PSUM"))

            # ---- weights (resident)
            w9 = wp.tile([128, 2, npe, 9, 128], BF16, tag="w9")
            G.dma_start(w9[:].rearrange("p a b c d -> p (a b c d)"), wdiag_d[:, :])
            wsc = wp.tile([128, 2, nsc, 9], F32, tag="wsc")
            bb = wp.tile([128, 2, len(BIAS_KEYS)], F32, tag="bb")
            for cb in range(2):
                sl = slice(cb * 128, (cb + 1) * 128)
                nc.sync.dma_start(
                    wsc[:, cb, :, :].rearrange("p a b -> p (a b)"), wsc_d[sl, :]
                )
                nc.sync.dma_start(bb[:, cb, :], bias_d[sl, :])

            def bias_ap(cb, key):
                k = BIAS_KEYS.index(key)
                return bb[:, cb, k : k + 1]

            def wsc_s(key, t, cb):
                k = SC_CONVS.index(key)
                return wsc[:, cb, k, t : t + 1]

            state = {}

            def pe_conv(cb, key, evict_dst, func, bias_key, planes):
                """stride-2 depthwise conv on PE (diagonal matmuls, TILE4),
                evicted to evict_dst by ACT with bias+func fused."""
                ee, eo, oe, oo = planes
                cvi = PE_CONVS.index(key)
                for h in range(2):
                    acc = ps.tile([128, 2048], F32, tag="ph", name="ph")
                    a3 = acc[:].rearrange("p (r c) -> p r c", r=32)
                    for c in range(4):
                        r0c = 32 * h + 8 * c
                        for t, (di, dj) in enumerate(TAPS9):
                            pl = eo if di == 1 else oo
                            if dj == 1:
                                pl = ee if di == 1 else oe
                            # rows
                            ro = -1 if di == 0 else 0
                            r0, nr, oro = r0c, 8, 0
                            if di == 0 and r0c == 0:
                                r0, nr, oro = 1, 7, 1
                            # cols: dj==0 reads odd col c-1 -> shift out +1
                            if dj == 0:
                                mv = pl[:, r0 + ro : r0 + ro + nr, 0:63]
                                ov = a3[:, 8 * c + oro : 8 * c + 8, 1:64]
                            else:
                                mv = pl[:, r0 + ro : r0 + ro + nr, 0:64]
                                ov = a3[:, 8 * c + oro : 8 * c + 8, 0:64]
                            for g in range(4):
                                gs = slice(32 * g, 32 * g + 32)
                                nc.tensor.matmul(
                                    ov[gs],
                                    w9[gs, cb, cvi, t, gs],
                                    mv[gs],
                                    start=(t == 0),
                                    stop=(t == 8),
                                    tile_position=(32 * g, 32 * g),
                                )
                    S.activation(
                        evict_dst[:, 32 * h : 32 * h + 32, :],
                        a3,
                        func,
                        bias=bias_ap(cb, bias_key),
                    )

            def dve_conv(cb, key, dst, bias_key, planes, part, act_taps=()):
                """stride-2 depthwise conv on DVE/ACT (scalar products + adds).

                part=0: center tap (bias) + the dj!=0 taps (aligned; products
                        on DVE tensor_scalar 4x unless tap in act_taps).
                part=1: the dj==0 (column-shift) taps — ACT products into a
                        zero-col0 tmp so DVE adds stay full-width 2x.
                """
                ee, eo, oe, oo = planes
                if part == 0:
                    V.tensor_scalar(
                        dst[:], ee[:], wsc_s(key, 4, cb), bias_ap(cb, bias_key),
                        OP.mult, OP.add,
                    )
                for t, (di, dj) in enumerate(TAPS9):
                    if di == 1 and dj == 1:
                        continue
                    if (dj == 0) != (part == 1):
                        continue
                    pl = eo if di == 1 else oo
                    if dj == 1:
                        pl = ee if di == 1 else oe
                    i0 = 1 if di == 0 else 0
                    rs_in = slice(0, 64 - i0)
                    rs_out = slice(i0, 64)
                    t_ = tmp.tile([128, 64, 64], BF16, tag="t", name="t")
                    if dj == 0:
                        G.memset(t_[:, rs_out, 0:1], 0)
                        S.mul(
                            t_[:, rs_out, 1:64],
                            pl[:, rs_in, 0:63],
                            wsc_s(key, t, cb),
                        )
                    elif t in act_taps:
                        S.mul(t_[:, rs_out, :], pl[:, rs_in, 0:64], wsc_s(key, t, cb))
                    else:
                        V.tensor_scalar(
                            t_[:, rs_out, :], pl[:, rs_in, 0:64],
                            wsc_s(key, t, cb), None, OP.mult,
                        )
                    V.tensor_tensor(
                        dst[:, rs_out, :], dst[:, rs_out, :], t_[:, rs_out, :],
                        OP.add,
                    )

            def stage_a(b, cb):
                sl = slice(cb * 128, (cb + 1) * 128)

                ee = pp.tile([128, 64, 64], BF16, tag="ee", name="ee")
                eo = pp.tile([128, 64, 64], BF16, tag="eo", name="eo")
                oe = pp.tile([128, 64, 64], BF16, tag="oe", name="oe")
                oo = pp.tile([128, 64, 64], BF16, tag="oo", name="oo")
                planes = (ee, eo, oe, oo)

                rows = H // NH  # 16 input rows per chunk -> 8 phase rows
                for h in range(NH):
                    Xh = xhp.tile([128, rows, 128], BF16, tag="Xh", name="Xh")
                    G.dma_start(
                        Xh[:].rearrange("p a b -> p (a b)"),
                        x_d[b, sl, h * rows * 128 : (h + 1) * rows * 128],
                    )
                    hs = slice(rows // 2 * h, rows // 2 * (h + 1))
                    S.copy(ee[:, hs, :], Xh[:, 0:rows:2, 0:128:2])
                    S.copy(eo[:, hs, :], Xh[:, 0:rows:2, 1:128:2])
                    G.tensor_copy(oe[:, hs, :], Xh[:, 1:rows:2, 0:128:2])
                    G.tensor_copy(oo[:, hs, :], Xh[:, 1:rows:2, 1:128:2])

                # ---- DVE conv cm: aligned taps first (feeds DVE while the
                # ACT queue drains copies before g0's eviction needs it)
                cmt = stp.tile([128, 64, 64], BF16, tag="cm", name="cm")
                n0z = stp.tile([128, 66, 68], BF16, tag="n0z", name="n0z")
                G.memset(n0z[:, 0:1, :], 0)
                G.memset(n0z[:, 65:66, :], 0)
                G.memset(n0z[:, 1:65, 1:2], 0)
                G.memset(n0z[:, 1:65, 66:67], 0)
                dve_conv(cb, "mb", cmt, "mb", planes, part=0)

                g0t = gp.tile([128, 64, 64], BF16, tag="g0", name="g0t", bufs=1)
                g1t = gp.tile([128, 64, 64], BF16, tag="g1", name="g1t", bufs=2)
                d01t = dtp.tile([128, 64, 64], BF16, tag="dt", name="d01t")
                p1t = dtp.tile([128, 64, 64], BF16, tag="dt", name="p1t")
                d23t = dtp.tile([128, 64, 64], BF16, tag="dt", name="d23t")
                p3t = stp.tile([128, 64, 64], BF16, tag="p3", name="p3t")

                pe_conv(cb, "g0", g0t, AF.Sigmoid, "g0", planes)
                dve_conv(cb, "mb", cmt, "mb", planes, part=1)
                pe_conv(cb, "d01", d01t, AF.Identity, "d01", planes)
                dve_conv(cb, "p3", p3t, "p3", planes, part=0, act_taps=(2,))
                pe_conv(cb, "p1", p1t, AF.Identity, "p1", planes)
                # e01 / n0 as soon as d01t+p1t exist (frees dtp bufs early)
                V.tensor_tensor(g0t[:], g0t[:], d01t[:], OP.mult)  # e01
                V.tensor_tensor(n0z[:, 1:65, 2:66], g0t[:], p1t[:], OP.add)
                pe_conv(cb, "g1", g1t, AF.Sigmoid, "g1", planes)
                dve_conv(cb, "p3", p3t, "p3", planes, part=1)
                pe_conv(cb, "d23", d23t, AF.Identity, "d23", planes)

                # ---- separable maxpool (in-place in m1e)
                m1e = tmp.tile([128, 64, 64], BF16, tag="t", name="m1e")
                m1o = tmp.tile([128, 64, 64], BF16, tag="t", name="m1o")
                V.tensor_tensor(m1e[:], ee[:], eo[:], OP.max)
                V.tensor_tensor(m1o[:], oe[:], oo[:], OP.max)
                V.tensor_tensor(
                    m1e[:, :, 1:64], m1e[:, :, 1:64], eo[:, :, 0:63], OP.max
                )
                V.tensor_tensor(
                    m1o[:, :, 1:64], m1o[:, :, 1:64], oo[:, :, 0:63], OP.max
                )
                V.tensor_tensor(m1e[:], m1e[:], m1o[:], OP.max)
                V.tensor_tensor(
                    m1e[:, 1:64, :], m1e[:, 1:64, :], m1o[:, 0:63, :], OP.max
                )
                V.tensor_tensor(cmt[:], cmt[:], m1e[:], OP.mult)  # mpcm

                # ---- combine: n1
                V.tensor_tensor(g1t[:], g1t[:], d23t[:], OP.mult)  # e23
                V.tensor_tensor(p3t[:], p3t[:], g1t[:], OP.add)  # n1

                state[(b, cb)] = dict(n0z=n0z, n1=p3t, mpcm=cmt, g1t=g1t)

            def stage_b(b, cb):
                sl = slice(cb * 128, (cb + 1) * 128)
                st = state.pop((b, cb))
                n0z, n1, mpcm = st["n0z"], st["n1"], st["mpcm"]
                g1t = st["g1t"]

                g1i = PE_CONVS.index("g1")
                gct = dtp.tile([128, 64, 64], BF16, tag="dt", name="gct")
                for h in range(2):
                    acc = ps.tile([128, 2048], F32, tag="ph", name="ph")
                    a3 = acc[:].rearrange("p (r c) -> p r c", r=32)
                    for c in range(4):
                        r0 = 32 * h + 8 * c
                        for t, (di, dj) in enumerate(TAPS9):
                            for g in range(4):
                                gs = slice(32 * g, 32 * g + 32)
                                nc.tensor.matmul(
                                    a3[gs, 8 * c : 8 * c + 8, :],
                                    w9[gs, cb, g1i, t, gs],
                                    n0z[gs, r0 + di : r0 + di + 8, dj + 1 : dj + 65],
                                    start=(t == 0),
                                    stop=(t == 8),
                                    tile_position=(32 * g, 32 * g),
                                )
                    S.activation(
                        gct[:, 32 * h : 32 * h + 32, :],
                        a3,
                        AF.Sigmoid,
                        bias=bias_ap(cb, "gc"),
                    )

                n0 = n0z[:, 1:65, 2:66]
                V.tensor_tensor(g1t[:], n0, n1[:], OP.subtract)
                V.tensor_tensor(g1t[:], g1t[:], gct[:], OP.mult)
                V.tensor_tensor(g1t[:], g1t[:], n1[:], OP.add)
                V.tensor_tensor(g1t[:], g1t[:], mpcm[:], OP.add)
                G.dma_start(out_d[b, sl, :], g1t[:].rearrange("p a b -> p (a b)"))

            planes_l = [(b, cb) for b in range(BS) for cb in range(2)]
            rep_ctx = tc.For_i(0, reps, 1) if reps > 1 else contextlib.nullcontext()
            with rep_ctx:
                stage_a(*planes_l[0])
                for i in range(1, len(planes_l)):
                    stage_a(*planes_l[i])
                    stage_b(*planes_l[i - 1])
                stage_b(*planes_l[-1])

    nc.compile()
    return nc


_NC_CACHE = {}


def _get_nc(reps=1):
    key = reps
    if key not in _NC_CACHE:
        _NC_CACHE[key] = _build(reps)
    return _NC_CACHE[key]


def _prep_weights(maxgate, mb, pconvs, pbs, pgates, gbs):
    npe = len(PE_CONVS)
    mg = np.asarray(maxgate, np.float32).reshape(C, 9)
    pc = np.asarray(pconvs, np.float32).reshape(C, 9, 4)
    pg = np.asarray(pgates, np.float32).reshape(C, 9, 3)
    pbs = np.asarray(pbs, np.float32)
    gbs = np.asarray(gbs, np.float32)
    mb = np.asarray(mb, np.float32).reshape(C)

    wconvs = {
        "mb": mg,
        "g0": pg[:, :, 0],
        "d01": pc[:, :, 0] - pc[:, :, 1],
        "p1": pc[:, :, 1],
        "g1": pg[:, :, 2],
        "d23": pc[:, :, 2] - pc[:, :, 3],
        "p3": pc[:, :, 3],
    }
    wd = np.zeros((128, 2, npe, 9, 128), np.float32)
    idx = np.arange(128)
    for cb in range(2):
        for cvi, key in enumerate(PE_CONVS):
            wd[idx, cb, cvi, :, idx] = wconvs[key][cb * 128 : (cb + 1) * 128, :]
    wsc = np.stack([wconvs[k] for k in SC_CONVS], axis=1)  # [C, nsc, 9]
    biases = np.stack(
        [
            mb,
            gbs[:, 0],
            pbs[:, 0] - pbs[:, 1],
            pbs[:, 1],
            gbs[:, 1],
            pbs[:, 2] - pbs[:, 3],
            pbs[:, 3],
            gbs[:, 2],
        ],
        axis=1,
    ).astype(np.float32)
    return (
        wd.reshape(128, 2 * npe * 9 * 128),
        wsc.reshape(C, len(SC_CONVS) * 9).astype(np.float32),
        biases,
    )


def _in_maps(x, maxgate, mb, pconvs, pbs, pgates, gbs):
    x = np.ascontiguousarray(np.asarray(x, np.float32))
    wd, wsc, biases = _prep_weights(maxgate, mb, pconvs, pbs, pgates, gbs)
    maps = []
    for i in range(N_CORES):
        maps.append(
            dict(
                x=x[i * BS : (i + 1) * BS].reshape(BS, C, H * H),
                wdiag=wd,
                wsc=wsc,
                biases=biases,
            )
        )
    return maps


def kernel(x, maxgate, mb, pconvs, pbs, pgates, gbs):
    nc = _get_nc(1)
    maps = _in_maps(x, maxgate, mb, pconvs, pbs, pgates, gbs)
    res = run_bass_kernel_spmd(nc, maps, list(range(N_CORES)))
    return np.concatenate(
        [r["out"].reshape(BS, C, HO, HO) for r in res.results], axis=0
    )
